# revision 22
# baseline (speedup 1.0000x reference)
"""Trainium2 Bass kernel for CPELayer_ResAG (concept-routed LoRA edit layer).

Computation (per token t with concept c = concept_idx[t]):
    down = edit_direction[t] @ lora_down[c]          # [768]@[768,4] -> [4]
    up   = down @ lora_up[c]                         # [4]@[4,1280]  -> [1280]
    out  = x[t] @ W.T + b_lin + 0.25 * up

Strategy: data-parallel over batch across 8 cores (616 tokens/core, padded
to 640 so every block is 128 wide).  The routed LoRA is computed densely:
A.T[(c,r), t] = lora_down_flat.T @ ed.T for ALL concepts, masked on-device
with a one-hot built by DVE is_equal (the MoE routing), and contracted back
with lora_up_flat on the tensor engine, accumulating into the same PSUM as
the org matmul.  The bias is folded in as one extra contraction row (ones
row in the masked operand, b_lin row in the lora_up operand).  The 0.25
LoRA scale is folded into lora_up host-side (exact: power of two).

Precision/throughput split (rel-err budget 2e-2, achieved ~1.7e-2):
  - org matmul k-tiles 0-1 (256 of 768 contraction rows): fp8e4m3 with one
    DoubleRow matmul per n-chunk half -- 2x PE rate and half the bytes.
  - org k-tiles 2-5: bf16 operands.
  - LoRA branch (ed/ld/lu/MT): fp8e4m3 DoubleRow throughout (the branch is
    ~0.7% of the output scale; fp8 error there is negligible).
  - output: bf16 on device, widened to f32 on host (layout-only).

All fp8 DoubleRow stationaries are 128 columns wide with k-pairs contiguous
in SBUF (ISA requirement); the host pre-swizzles every operand so each DMA
is a plain [128 x contiguous-bytes] copy (the strided-gather DMAs of the
previous revision ran at ~270 GB/s; plain rows run at ~400 GB/s).

Wave structure per t-block (PSUM banks recycle during the load phase):
  wave A: org k0k1 DoubleRow + k2,k3 bf16 -> copy to f32 staging
  wave B: up-projection DoubleRow + k4,k5 bf16 -> DVE add -> bf16 out DMA
k-outer ordering inside each wave keeps the PE stationary operand constant
across the n-chunks (fewer weight reloads).
"""

import sys
import types

import numpy as np

import concourse.mybir as mybir
import concourse.tile as tile
from concourse import bacc
from concourse.bass_utils import run_bass_kernel_spmd

# If BASS_TRACE is set in the environment, run_bass_kernel_spmd imports
# antenv.axon_hooks, which some containers lack; stub it (None hook ->
# tracing is skipped gracefully, execution unaffected).
try:
    import antenv.axon_hooks  # noqa: F401
except ImportError:
    _m = types.ModuleType("antenv.axon_hooks")
    _m.get_axon_ntff_profile_hook = lambda: None
    _m.set_axon_ntff_profile_hook = lambda h: None
    sys.modules["antenv.axon_hooks"] = _m

# Problem shapes (hardcoded per spec nn_CPELayer_ResAG_19335942766951)
N_CORES = 8
B, T, DIN, DOUT = 64, 77, 768, 1280
N_CONCEPTS, RANK = 50, 4
SCALE = 0.25  # alpha/rank = 1/4, exact power of two
BPC = B // N_CORES          # batches per core = 8
TOK = BPC * T               # tokens per core = 616
TOKP = 640                  # padded tokens (DoubleRow stationaries need
                            # 128-wide blocks; pad cols are masked/discarded)
NJ = N_CONCEPTS * RANK      # 200 flattened (concept, rank) rows
KJ_PAD = 256                # padded rows: 200 lora + 1 bias + 55 zero
P = 128
KD = DIN // P               # 6 k-tiles of the d_in contraction
KF = 2                      # leading k-tiles in fp8 DoubleRow (k0,k1)
NT = TOKP // P              # 5 t-blocks of 128 tokens
KP = KD // 2                # 3 DoubleRow k-pairs for the A.T matmul
N_CHUNKS = [(0, 512), (512, 512), (1024, 256)]
# A.T token chunks (DoubleRow moving free <= 512 => <=256 tokens per chunk),
# grouped into two PSUM tiles per concept j-tile: [256+256 | 128]
AT_TILES = [[(0, 256), (256, 256)], [(512, 128)]]

_cache = {}


def _build_bass():
    nc = bacc.Bacc("TRN2", target_bir_lowering=False, debug=False,
                   num_devices=N_CORES)
    f32 = mybir.dt.float32
    bf16 = mybir.dt.bfloat16
    fp8 = mybir.dt.float8e4

    # Pre-swizzled host layouts: every tensor is [128, row-bytes] with the
    # SBUF tile's free dims flattened along the row.
    idx_d = nc.dram_tensor("idxf", [1, TOKP], f32, kind="ExternalInput").ap()
    cv_d = nc.dram_tensor("cvals", [P, 2], f32, kind="ExternalInput").ap()
    ld_d = nc.dram_tensor("ldH", [P, 2 * KD * P], fp8,
                          kind="ExternalInput").ap()
    ed_d = nc.dram_tensor("edH", [P, KP * 2 * TOKP], fp8,
                          kind="ExternalInput").ap()
    x8_d = nc.dram_tensor("x8H", [P, NT * KF * P], fp8,
                          kind="ExternalInput").ap()
    W8_d = nc.dram_tensor("W8H", [P, KF * DOUT], fp8,
                          kind="ExternalInput").ap()
    lu_d = nc.dram_tensor("luH", [P, 2 * DOUT], fp8,
                          kind="ExternalInput").ap()
    xT_d = nc.dram_tensor("xT", [DIN, TOKP], bf16, kind="ExternalInput").ap()
    WT_d = nc.dram_tensor("WT", [DIN, DOUT], bf16, kind="ExternalInput").ap()
    out_d = nc.dram_tensor("out", [TOKP, DOUT], bf16,
                           kind="ExternalOutput").ap()

    with tile.TileContext(nc) as tc:
        with (
            tc.tile_pool(name="consts", bufs=1) as consts,
            tc.tile_pool(name="outsb", bufs=10) as outsb,
        ):
            # Load order tracks the consumption order: the A.T chain (ld/ed)
            # first, then the wave-A org operands (x8/W8, xT/WT k2-3), the
            # up-matmul rhs (lu), and the wave-B org tail (k4-5).
            # Single Sync-queue FIFO in consumption order: the DMA
            # engines drain multiple queues round-robin, so a second queue
            # steals bandwidth from the critical chain rather than
            # prioritizing it (measured).  ld/ed lead (the A.T chain), then
            # the wave-A org operands, the routing tensors, and the wave-B
            # tail.  ed arrives as 3 k-pair slices so the first A.T matmuls
            # start ~1us earlier.
            ld_jc = []
            for jc in range(2):
                t_ = consts.tile([P, KD, P], fp8, tag=f"ld{jc}")
                nc.sync.dma_start(t_[:],
                                  ld_d[:, jc * KD * P:(jc + 1) * KD * P]
                                  .rearrange("p (k j) -> p k j", k=KD))
                ld_jc.append(t_)
            ed_all = consts.tile([P, KP, 2, TOKP], fp8, tag="ed_all")
            nc.sync.dma_start(ed_all[:],
                              ed_d.rearrange("p (i h t) -> p i h t",
                                             i=KP, h=2))
            x8 = consts.tile([P, NT, KF, P], fp8, tag="x8")
            nc.sync.dma_start(x8[:],
                              x8_d.rearrange("p (t h u) -> p t h u",
                                             t=NT, h=KF))
            W8 = consts.tile([P, KF, DOUT], fp8, tag="W8")
            nc.sync.dma_start(W8[:],
                              W8_d.rearrange("p (h o) -> p h o", h=KF))
            xT = {}
            WT = {}

            def load_kpair(k):
                t_ = consts.tile([P, TOKP], bf16, tag=f"xT{k}")
                nc.sync.dma_start(t_[:], xT_d[k * P:(k + 1) * P, :])
                xT[k] = t_
                t_ = consts.tile([P, DOUT], bf16, tag=f"WT{k}")
                nc.sync.dma_start(t_[:], WT_d[k * P:(k + 1) * P, :])
                WT[k] = t_

            load_kpair(2)
            load_kpair(3)
            cvals = consts.tile([P, 2], f32, tag="cvals")
            nc.sync.dma_start(cvals[:], cv_d[:, :])
            idx_bc = consts.tile([P, TOKP], f32, tag="idx_bc")
            nc.sync.dma_start(idx_bc[:], idx_d.partition_broadcast(P))
            lu_all = consts.tile([P, 2, DOUT], fp8, tag="lu_all")
            nc.sync.dma_start(lu_all[:],
                              lu_d.rearrange("p (j o) -> p j o", j=2))
            load_kpair(4)
            load_kpair(5)

            masks = []
            for jc in range(2):
                m = consts.tile([P, TOKP], f32, tag=f"mask{jc}")
                nc.vector.tensor_scalar(
                    m[:], idx_bc[:], cvals[:, jc:jc + 1], None,
                    mybir.AluOpType.is_equal)
                masks.append(m)

            # MT[(c,r) row, j-tile, t]: the routed "down" activations,
            # transposed, in fp8 for the DoubleRow up-projection.  One tile
            # per t-block so the stationary k-pair slice is contiguous.
            # j-tile 1 rows 72..127 pair with luB rows 200..255: zero them,
            # then the ones row at 96 (bias: b_lin sits at luB[224]); the
            # mask-mul below overwrites rows 0..71 (lora j=128..199).
            MTb = []
            for ti in range(NT):
                mt = consts.tile([P, 2, P], fp8, tag=f"MT{ti}")
                nc.vector.memset(mt[64:P, 1, :], 0.0)
                nc.vector.memset(mt[96:97, 1, :], 1.0)
                MTb.append(mt)

            # A.T[(c,r), t] = lora_down_flat.T @ ed.T for all concepts via
            # fp8 DoubleRow (2 k-tiles per matmul), masked into MT.
            with tc.tile_pool(name="at_ps", bufs=4, space="PSUM") as at_pool:
                for jc in range(2):
                    # matmul always 128 rows (ld zero-padded); the mask-mul
                    # only writes the 72 real lora rows of j-chunk 1 so the
                    # memset bias/zero rows survive.
                    mjp = P if jc == 0 else NJ - P  # 128, 72
                    for chunks in AT_TILES:
                        at = at_pool.tile([P, 512], f32, tag="at")
                        base = chunks[0][0]
                        ni = 0
                        nmm = len(chunks) * KP
                        for i in range(KP):
                            for (n0, nw) in chunks:
                                nc.tensor.matmul(
                                    at[:, n0 - base:n0 - base + nw],
                                    ld_jc[jc][:, 2 * i:2 * i + 2, :],
                                    ed_all[:, i, :, n0:n0 + nw],
                                    start=(ni == 0), stop=(ni == nmm - 1),
                                    perf_mode=mybir.MatmulPerfMode.DoubleRow)
                                ni += 1
                        cw = sum(nw for _, nw in chunks)
                        # scatter the masked rows into the per-t-block MT
                        # tiles covered by this psum tile ([0:512] spans
                        # t-blocks 0-3; [512:640] is exactly t-block 4)
                        for ti in range(NT):
                            t0, t1 = ti * P, (ti + 1) * P
                            if t0 < base or t1 > base + cw:
                                continue
                            nc.vector.tensor_tensor(
                                MTb[ti][:mjp, jc, :],
                                at[:mjp, t0 - base:t1 - base],
                                masks[jc][:mjp, t0:t1],
                                mybir.AluOpType.mult)

            # Main accumulation, two short-lived PSUM waves per (t, n) so
            # banks recycle during the load phase:
            #   wave A: org k0k1 (one fp8 DoubleRow per 256-half) + k2,k3
            #           bf16 -> copy to f32 staging
            #   wave B: up-projection DoubleRow + org k4,k5 -> DVE add ->
            #           bf16 out DMA
            with tc.tile_pool(name="out_ps", bufs=8, space="PSUM") as out_pool:
                osbs = []
                for ti in range(NT):
                    t0 = ti * P
                    tsl = slice(t0, t0 + P)
                    osb = outsb.tile([P, DOUT], f32, tag="osb")
                    osbs.append(osb)
                    pss = []
                    for _ci in range(len(N_CHUNKS)):
                        ps = out_pool.tile([P, 512], f32, tag="ops")
                        pss.append(ps)
                    for ci, (n0, nw) in enumerate(N_CHUNKS):
                        for h0 in range(0, nw, 256):
                            nc.tensor.matmul(
                                pss[ci][:, h0:h0 + 256],
                                x8[:, ti, :, :],
                                W8[:, :, n0 + h0:n0 + h0 + 256],
                                start=(h0 == 0), stop=False,
                                perf_mode=mybir.MatmulPerfMode.DoubleRow)
                    for k in (2, 3):
                        for ci, (n0, nw) in enumerate(N_CHUNKS):
                            nc.tensor.matmul(
                                pss[ci][:, :nw], xT[k][:, tsl],
                                WT[k][:, n0:n0 + nw],
                                start=False, stop=(k == 3))
                    for ci, (n0, nw) in enumerate(N_CHUNKS):
                        nc.any.tensor_copy(out=osb[:, n0:n0 + nw],
                                           in_=pss[ci][:, :nw])
                for ti in range(NT):
                    t0 = ti * P
                    tsl = slice(t0, t0 + P)
                    osb = osbs[ti]
                    obb = outsb.tile([P, DOUT], bf16, tag="obb")
                    pss = []
                    for _ci in range(len(N_CHUNKS)):
                        ps = out_pool.tile([P, 512], f32, tag="ops")
                        pss.append(ps)
                    for ci, (n0, nw) in enumerate(N_CHUNKS):
                        for h0 in range(0, nw, 256):
                            nc.tensor.matmul(
                                pss[ci][:, h0:h0 + 256],
                                MTb[ti][:, :, :],
                                lu_all[:, :, n0 + h0:n0 + h0 + 256],
                                start=(h0 == 0), stop=False,
                                perf_mode=mybir.MatmulPerfMode.DoubleRow)
                    for k in (4, 5):
                        for ci, (n0, nw) in enumerate(N_CHUNKS):
                            nc.tensor.matmul(
                                pss[ci][:, :nw], xT[k][:, tsl],
                                WT[k][:, n0:n0 + nw],
                                start=False, stop=(k == 5))
                    if ti < NT - 1:
                        for ci, (n0, nw) in enumerate(N_CHUNKS):
                            nc.vector.tensor_tensor(
                                obb[:, n0:n0 + nw], pss[ci][:, :nw],
                                osb[:, n0:n0 + nw], mybir.AluOpType.add)
                        nc.sync.dma_start(out_d[tsl, :], obb[:, :])
                    else:
                        # last t-block: chunked add->DMA pipeline shortens
                        # the endgame tail (the DMA of chunk 0 streams while
                        # chunk 1 is still being added)
                        for ci, (n0, nw) in enumerate(N_CHUNKS):
                            nc.vector.tensor_tensor(
                                obb[:, n0:n0 + nw], pss[ci][:, :nw],
                                osb[:, n0:n0 + nw], mybir.AluOpType.add)
                            nc.sync.dma_start(out_d[tsl, n0:n0 + nw],
                                              obb[:, n0:n0 + nw])

    nc.compile()
    return nc


def get_bass():
    if "nc" not in _cache:
        _cache["nc"] = _build_bass()
    return _cache["nc"]


def make_in_maps(x, edit_direction, concept_idx, lora_down, lora_up, W, b_lin):
    """Host-side sharding + layout/dtype prep (no reference FLOPs)."""
    import ml_dtypes
    bf16 = ml_dtypes.bfloat16
    fp8 = ml_dtypes.float8_e4m3

    x = np.asarray(x, dtype=np.float32)
    ed = np.asarray(edit_direction, dtype=np.float32)
    idx = np.asarray(concept_idx)
    ld = np.asarray(lora_down, dtype=np.float32)
    lup = np.asarray(lora_up, dtype=np.float32)
    W = np.asarray(W, dtype=np.float32)
    b = np.asarray(b_lin, dtype=np.float32)

    WTf = np.ascontiguousarray(W.T)                             # [768, 1280]
    WT = WTf.astype(bf16)
    # W8H[p, h*1280+o] = W.T[h*128+p, o] for k-tiles h in {0,1}
    W8H = np.ascontiguousarray(
        WTf[:KF * P].reshape(KF, P, DOUT).transpose(1, 0, 2)
        .reshape(P, KF * DOUT).astype(fp8))

    # ldH[p, jc*768 + k*128 + j] = lora_down_flat[k*128+p, jc*128+j]
    ldT = np.zeros((DIN, KJ_PAD), dtype=np.float32)
    ldT[:, :NJ] = ld.transpose(1, 0, 2).reshape(DIN, NJ)
    ldH = np.ascontiguousarray(
        ldT.reshape(KD, P, 2, P).transpose(1, 2, 0, 3)
        .reshape(P, 2 * KD * P).astype(fp8))

    # luH[p, j*1280+o] = luB[j*128+p, o]
    luB = np.zeros((KJ_PAD, DOUT), dtype=np.float32)
    luB[:NJ] = lup.reshape(NJ, DOUT) * SCALE                    # exact x0.25
    luB[128 + 96] = b                                           # bias row
    luH = np.ascontiguousarray(
        luB.reshape(2, P, DOUT).transpose(1, 0, 2)
        .reshape(P, 2 * DOUT).astype(fp8))

    cv = np.full(2 * P, -1.0, dtype=np.float32)
    cv[:NJ] = np.arange(NJ, dtype=np.float32) // RANK
    cvals = np.ascontiguousarray(cv.reshape(2, P).T)            # [128, 2]

    in_maps = []
    for c in range(N_CORES):
        sl = slice(c * BPC, (c + 1) * BPC)
        xs = np.zeros((TOKP, DIN), dtype=np.float32)
        xs[:TOK] = x[sl].reshape(TOK, DIN)
        eds = np.zeros((TOKP, DIN), dtype=np.float32)
        eds[:TOK] = ed[sl].reshape(TOK, DIN)
        idxs = np.full(TOKP, -1.0, dtype=np.float32)
        idxs[:TOK] = idx[sl].reshape(TOK).astype(np.float32)
        xsT = np.ascontiguousarray(xs.T)                        # [768, 640]
        edT = np.ascontiguousarray(eds.T)
        # edH[p, i*1280 + h*640 + t] = ed.T[(2i+h)*128+p, t]
        edH = np.ascontiguousarray(
            edT.reshape(KP, 2, P, TOKP).transpose(2, 0, 1, 3)
            .reshape(P, KP * 2 * TOKP).astype(fp8))
        # x8H[p, ti*256 + h*128 + u] = x.T[h*128+p, ti*128+u]
        x8H = np.ascontiguousarray(
            xsT[:KF * P].reshape(KF, P, NT, P).transpose(1, 2, 0, 3)
            .reshape(P, NT * KF * P).astype(fp8))
        in_maps.append({
            "xT": np.ascontiguousarray(xsT.astype(bf16)),
            "x8H": x8H,
            "edH": edH,
            "idxf": np.ascontiguousarray(idxs.reshape(1, TOKP)),
            "cvals": cvals,
            "WT": WT,
            "W8H": W8H,
            "ldH": ldH,
            "luH": luH,
        })
    return in_maps


def kernel(x, edit_direction, concept_idx, lora_down, lora_up, W, b_lin,
           _trace=False, **_ignored):
    nc = get_bass()
    in_maps = make_in_maps(x, edit_direction, concept_idx, lora_down, lora_up,
                           W, b_lin)
    res = run_bass_kernel_spmd(nc, in_maps, core_ids=list(range(N_CORES)),
                               trace=_trace)
    out = np.concatenate([np.asarray(r["out"][:TOK], dtype=np.float32)
                          for r in res.results], axis=0)
    out = out.reshape(B, T, DOUT)
    if _trace:
        kernel.last_results = res
    return out


# revision 23
# speedup vs baseline: 1.1286x; 1.1286x over previous
"""Trainium2 Bass kernel for CPELayer_ResAG (concept-routed LoRA edit layer).

Computation (per token t with concept c = concept_idx[t]):
    down = edit_direction[t] @ lora_down[c]          # [768]@[768,4] -> [4]
    up   = down @ lora_up[c]                         # [4]@[4,1280]  -> [1280]
    out  = x[t] @ W.T + b_lin + 0.25 * up

Strategy: data-parallel over batch across 8 cores (616 tokens/core, padded
to 640 so every block is 128 wide).  The routed LoRA is computed densely:
A.T[(c,r), t] = lora_down_flat.T @ ed.T for ALL concepts, masked on-device
with a one-hot built by DVE is_equal (the MoE routing), and contracted back
with lora_up_flat on the tensor engine, accumulating into the same PSUM as
the org matmul.  The bias is folded in as one extra contraction row (ones
row in the masked operand, b_lin row in the lora_up operand).  The 0.25
LoRA scale is folded into lora_up host-side (exact: power of two).

Precision/throughput split (rel-err budget 2e-2, achieved ~1.7e-2):
  - org matmul k-tiles 0-1 (256 of 768 contraction rows): fp8e4m3 with one
    DoubleRow matmul per n-chunk half -- 2x PE rate and half the bytes.
  - org k-tiles 2-5: bf16 operands.
  - LoRA branch (ed/ld/lu/MT): fp8e4m3 DoubleRow throughout (the branch is
    ~0.7% of the output scale; fp8 error there is negligible).
  - output: bf16 on device, widened to f32 on host (layout-only).

All fp8 DoubleRow stationaries are 128 columns wide with k-pairs contiguous
in SBUF (ISA requirement); the host pre-swizzles every operand so each DMA
is a plain [128 x contiguous-bytes] copy (the strided-gather DMAs of the
previous revision ran at ~270 GB/s; plain rows run at ~400 GB/s).

Wave structure per t-block (PSUM banks recycle during the load phase):
  wave A: org k0k1 DoubleRow + k2,k3 bf16 -> copy to f32 staging
  wave B: up-projection DoubleRow + k4,k5 bf16 -> DVE add -> bf16 out DMA
k-outer ordering inside each wave keeps the PE stationary operand constant
across the n-chunks (fewer weight reloads).
"""

import sys
import types

import numpy as np

import concourse.mybir as mybir
import concourse.tile as tile
from concourse import bacc
from concourse.bass_utils import run_bass_kernel_spmd

# If BASS_TRACE is set in the environment, run_bass_kernel_spmd imports
# antenv.axon_hooks, which some containers lack; stub it (None hook ->
# tracing is skipped gracefully, execution unaffected).
try:
    import antenv.axon_hooks  # noqa: F401
except ImportError:
    _m = types.ModuleType("antenv.axon_hooks")
    _m.get_axon_ntff_profile_hook = lambda: None
    _m.set_axon_ntff_profile_hook = lambda h: None
    sys.modules["antenv.axon_hooks"] = _m

# Problem shapes (hardcoded per spec nn_CPELayer_ResAG_19335942766951)
N_CORES = 8
B, T, DIN, DOUT = 64, 77, 768, 1280
N_CONCEPTS, RANK = 50, 4
SCALE = 0.25  # alpha/rank = 1/4, exact power of two
BPC = B // N_CORES          # batches per core = 8
TOK = BPC * T               # tokens per core = 616
TOKP = 640                  # padded tokens (DoubleRow stationaries need
                            # 128-wide blocks; pad cols are masked/discarded)
NJ = N_CONCEPTS * RANK      # 200 flattened (concept, rank) rows
KJ_PAD = 256                # padded rows: 200 lora + 1 bias + 55 zero
P = 128
KD = DIN // P               # 6 k-tiles of the d_in contraction
KF = 2                      # leading k-tiles in fp8 DoubleRow (k0,k1)
NT = TOKP // P              # 5 t-blocks of 128 tokens
KP = KD // 2                # 3 DoubleRow k-pairs for the A.T matmul
N_CHUNKS = [(0, 512), (512, 512), (1024, 256)]
# A.T token chunks (DoubleRow moving free <= 512 => <=256 tokens per chunk),
# grouped into two PSUM tiles per concept j-tile: [256+256 | 128]
AT_TILES = [[(0, 256), (256, 256)], [(512, 128)]]

_cache = {}


def _build_bass():
    nc = bacc.Bacc("TRN2", target_bir_lowering=False, debug=False,
                   num_devices=N_CORES)
    f32 = mybir.dt.float32
    bf16 = mybir.dt.bfloat16
    fp8 = mybir.dt.float8e4

    # Pre-swizzled host layouts: every tensor is [128, row-bytes] with the
    # SBUF tile's free dims flattened along the row.
    idx_d = nc.dram_tensor("idxf", [1, TOKP], f32, kind="ExternalInput").ap()
    cv_d = nc.dram_tensor("cvals", [P, 2], f32, kind="ExternalInput").ap()
    ld_d = nc.dram_tensor("ldH", [P, 2 * KD * P], fp8,
                          kind="ExternalInput").ap()
    ed_d = nc.dram_tensor("edH", [P, KP * 2 * TOKP], fp8,
                          kind="ExternalInput").ap()
    x8_d = nc.dram_tensor("x8H", [P, NT * KF * P], fp8,
                          kind="ExternalInput").ap()
    W8_d = nc.dram_tensor("W8H", [P, KF * DOUT], fp8,
                          kind="ExternalInput").ap()
    lu_d = nc.dram_tensor("luH", [P, 2 * DOUT], fp8,
                          kind="ExternalInput").ap()
    xT_d = nc.dram_tensor("xT", [DIN, TOKP], bf16, kind="ExternalInput").ap()
    WT_d = nc.dram_tensor("WT", [DIN, DOUT], bf16, kind="ExternalInput").ap()
    out_d = nc.dram_tensor("out", [TOKP, DOUT], bf16,
                           kind="ExternalOutput").ap()

    with tile.TileContext(nc) as tc:
        with (
            tc.tile_pool(name="consts", bufs=1) as consts,
            tc.tile_pool(name="outsb", bufs=10) as outsb,
            tc.tile_pool(name="at_ps", bufs=2, space="PSUM") as at_pool,
            tc.tile_pool(name="out_ps", bufs=6, space="PSUM") as out_pool,
        ):
            # Load order tracks the consumption order: the A.T chain (ld/ed)
            # first, then the wave-A org operands (x8/W8, xT/WT k2-3), the
            # up-matmul rhs (lu), and the wave-B org tail (k4-5).
            # Single Sync-queue FIFO in consumption order: the DMA
            # engines drain multiple queues round-robin, so a second queue
            # steals bandwidth from the critical chain rather than
            # prioritizing it (measured).  ld/ed lead (the A.T chain), then
            # the wave-A org operands, the routing tensors, and the wave-B
            # tail.  ed arrives as 3 k-pair slices so the first A.T matmuls
            # start ~1us earlier.
            ld_jc = []
            for jc in range(2):
                t_ = consts.tile([P, KD, P], fp8, tag=f"ld{jc}")
                nc.sync.dma_start(t_[:],
                                  ld_d[:, jc * KD * P:(jc + 1) * KD * P]
                                  .rearrange("p (k j) -> p k j", k=KD))
                ld_jc.append(t_)
            ed_all = consts.tile([P, KP, 2, TOKP], fp8, tag="ed_all")
            nc.sync.dma_start(ed_all[:],
                              ed_d.rearrange("p (i h t) -> p i h t",
                                             i=KP, h=2))
            cvals = consts.tile([P, 2], f32, tag="cvals")
            nc.sync.dma_start(cvals[:], cv_d[:, :])
            idx_bc = consts.tile([P, TOKP], f32, tag="idx_bc")
            nc.sync.dma_start(idx_bc[:], idx_d.partition_broadcast(P))
            x8 = consts.tile([P, NT, KF, P], fp8, tag="x8")
            nc.sync.dma_start(x8[:],
                              x8_d.rearrange("p (t h u) -> p t h u",
                                             t=NT, h=KF))
            W8 = consts.tile([P, KF, DOUT], fp8, tag="W8")
            nc.sync.dma_start(W8[:],
                              W8_d.rearrange("p (h o) -> p h o", h=KF))
            xT = {}
            WT = {}

            def load_kpair(k):
                t_ = consts.tile([P, TOKP], bf16, tag=f"xT{k}")
                nc.sync.dma_start(t_[:], xT_d[k * P:(k + 1) * P, :])
                xT[k] = t_
                t_ = consts.tile([P, DOUT], bf16, tag=f"WT{k}")
                nc.sync.dma_start(t_[:], WT_d[k * P:(k + 1) * P, :])
                WT[k] = t_

            load_kpair(2)
            load_kpair(3)
            lu_all = consts.tile([P, 2, DOUT], fp8, tag="lu_all")
            nc.sync.dma_start(lu_all[:],
                              lu_d.rearrange("p (j o) -> p j o", j=2))
            load_kpair(4)
            load_kpair(5)

            masks = []
            for jc in range(2):
                m = consts.tile([P, TOKP], f32, tag=f"mask{jc}")
                nc.vector.tensor_scalar(
                    m[:], idx_bc[:], cvals[:, jc:jc + 1], None,
                    mybir.AluOpType.is_equal)
                masks.append(m)

            # MT[(c,r) row, j-tile, t]: the routed "down" activations,
            # transposed, in fp8 for the DoubleRow up-projection.  One tile
            # per t-block so the stationary k-pair slice is contiguous.
            # j-tile 1 rows 72..127 pair with luB rows 200..255: zero them,
            # then the ones row at 96 (bias: b_lin sits at luB[224]); the
            # mask-mul below overwrites rows 0..71 (lora j=128..199).
            MTb = []
            for ti in range(NT):
                mt = consts.tile([P, 2, P], fp8, tag=f"MT{ti}")
                nc.vector.memset(mt[64:P, 1, :], 0.0)
                nc.vector.memset(mt[96:97, 1, :], 1.0)
                MTb.append(mt)

            # A.T[(c,r), t] = lora_down_flat.T @ ed.T for all concepts via
            # fp8 DoubleRow (2 k-tiles per matmul), masked into MT.
            # (at_ps coexists with out_ps -- 2+6 of 8 PSUM banks -- so wave
            # A needs no pool-transition barrier behind the mask-muls.)
            if True:
                for jc in range(2):
                    # matmul always 128 rows (ld zero-padded); the mask-mul
                    # only writes the 72 real lora rows of j-chunk 1 so the
                    # memset bias/zero rows survive.
                    mjp = P if jc == 0 else NJ - P  # 128, 72
                    for chunks in AT_TILES:
                        at = at_pool.tile([P, 512], f32, tag="at")
                        base = chunks[0][0]
                        ni = 0
                        nmm = len(chunks) * KP
                        for i in range(KP):
                            for (n0, nw) in chunks:
                                nc.tensor.matmul(
                                    at[:, n0 - base:n0 - base + nw],
                                    ld_jc[jc][:, 2 * i:2 * i + 2, :],
                                    ed_all[:, i, :, n0:n0 + nw],
                                    start=(ni == 0), stop=(ni == nmm - 1),
                                    perf_mode=mybir.MatmulPerfMode.DoubleRow)
                                ni += 1
                        cw = sum(nw for _, nw in chunks)
                        # scatter the masked rows into the per-t-block MT
                        # tiles covered by this psum tile ([0:512] spans
                        # t-blocks 0-3; [512:640] is exactly t-block 4)
                        for ti in range(NT):
                            t0, t1 = ti * P, (ti + 1) * P
                            if t0 < base or t1 > base + cw:
                                continue
                            nc.vector.tensor_tensor(
                                MTb[ti][:mjp, jc, :],
                                at[:mjp, t0 - base:t1 - base],
                                masks[jc][:mjp, t0:t1],
                                mybir.AluOpType.mult)

            # Main accumulation, two short-lived PSUM waves per (t, n) so
            # banks recycle during the load phase:
            #   wave A: org k0k1 (one fp8 DoubleRow per 256-half) + k2,k3
            #           bf16 -> copy to f32 staging
            #   wave B: up-projection DoubleRow + org k4,k5 -> DVE add ->
            #           bf16 out DMA
            if True:
                osbs = []
                for ti in range(NT):
                    t0 = ti * P
                    tsl = slice(t0, t0 + P)
                    osb = outsb.tile([P, DOUT], f32, tag="osb")
                    osbs.append(osb)
                    pss = []
                    for _ci in range(len(N_CHUNKS)):
                        ps = out_pool.tile([P, 512], f32, tag="ops")
                        pss.append(ps)
                    for ci, (n0, nw) in enumerate(N_CHUNKS):
                        for h0 in range(0, nw, 256):
                            nc.tensor.matmul(
                                pss[ci][:, h0:h0 + 256],
                                x8[:, ti, :, :],
                                W8[:, :, n0 + h0:n0 + h0 + 256],
                                start=(h0 == 0), stop=False,
                                perf_mode=mybir.MatmulPerfMode.DoubleRow)
                    for k in (2, 3):
                        for ci, (n0, nw) in enumerate(N_CHUNKS):
                            nc.tensor.matmul(
                                pss[ci][:, :nw], xT[k][:, tsl],
                                WT[k][:, n0:n0 + nw],
                                start=False, stop=(k == 3))
                    for ci, (n0, nw) in enumerate(N_CHUNKS):
                        nc.any.tensor_copy(out=osb[:, n0:n0 + nw],
                                           in_=pss[ci][:, :nw])
                for ti in range(NT):
                    t0 = ti * P
                    tsl = slice(t0, t0 + P)
                    osb = osbs[ti]
                    obb = outsb.tile([P, DOUT], bf16, tag="obb")
                    pss = []
                    for _ci in range(len(N_CHUNKS)):
                        ps = out_pool.tile([P, 512], f32, tag="ops")
                        pss.append(ps)
                    for ci, (n0, nw) in enumerate(N_CHUNKS):
                        for h0 in range(0, nw, 256):
                            nc.tensor.matmul(
                                pss[ci][:, h0:h0 + 256],
                                MTb[ti][:, :, :],
                                lu_all[:, :, n0 + h0:n0 + h0 + 256],
                                start=(h0 == 0), stop=False,
                                perf_mode=mybir.MatmulPerfMode.DoubleRow)
                    for k in (4, 5):
                        for ci, (n0, nw) in enumerate(N_CHUNKS):
                            nc.tensor.matmul(
                                pss[ci][:, :nw], xT[k][:, tsl],
                                WT[k][:, n0:n0 + nw],
                                start=False, stop=(k == 5))
                    if ti < NT - 1:
                        for ci, (n0, nw) in enumerate(N_CHUNKS):
                            nc.vector.tensor_tensor(
                                obb[:, n0:n0 + nw], pss[ci][:, :nw],
                                osb[:, n0:n0 + nw], mybir.AluOpType.add)
                        nc.sync.dma_start(out_d[tsl, :], obb[:, :])
                    else:
                        # last t-block: chunked add->DMA pipeline shortens
                        # the endgame tail (the DMA of chunk 0 streams while
                        # chunk 1 is still being added)
                        for ci, (n0, nw) in enumerate(N_CHUNKS):
                            nc.vector.tensor_tensor(
                                obb[:, n0:n0 + nw], pss[ci][:, :nw],
                                osb[:, n0:n0 + nw], mybir.AluOpType.add)
                            nc.sync.dma_start(out_d[tsl, n0:n0 + nw],
                                              obb[:, n0:n0 + nw])

    nc.compile()
    return nc


def get_bass():
    if "nc" not in _cache:
        _cache["nc"] = _build_bass()
    return _cache["nc"]


def make_in_maps(x, edit_direction, concept_idx, lora_down, lora_up, W, b_lin):
    """Host-side sharding + layout/dtype prep (no reference FLOPs)."""
    import ml_dtypes
    bf16 = ml_dtypes.bfloat16
    fp8 = ml_dtypes.float8_e4m3

    x = np.asarray(x, dtype=np.float32)
    ed = np.asarray(edit_direction, dtype=np.float32)
    idx = np.asarray(concept_idx)
    ld = np.asarray(lora_down, dtype=np.float32)
    lup = np.asarray(lora_up, dtype=np.float32)
    W = np.asarray(W, dtype=np.float32)
    b = np.asarray(b_lin, dtype=np.float32)

    WTf = np.ascontiguousarray(W.T)                             # [768, 1280]
    WT = WTf.astype(bf16)
    # W8H[p, h*1280+o] = W.T[h*128+p, o] for k-tiles h in {0,1}
    W8H = np.ascontiguousarray(
        WTf[:KF * P].reshape(KF, P, DOUT).transpose(1, 0, 2)
        .reshape(P, KF * DOUT).astype(fp8))

    # ldH[p, jc*768 + k*128 + j] = lora_down_flat[k*128+p, jc*128+j]
    ldT = np.zeros((DIN, KJ_PAD), dtype=np.float32)
    ldT[:, :NJ] = ld.transpose(1, 0, 2).reshape(DIN, NJ)
    ldH = np.ascontiguousarray(
        ldT.reshape(KD, P, 2, P).transpose(1, 2, 0, 3)
        .reshape(P, 2 * KD * P).astype(fp8))

    # luH[p, j*1280+o] = luB[j*128+p, o]
    luB = np.zeros((KJ_PAD, DOUT), dtype=np.float32)
    luB[:NJ] = lup.reshape(NJ, DOUT) * SCALE                    # exact x0.25
    luB[128 + 96] = b                                           # bias row
    luH = np.ascontiguousarray(
        luB.reshape(2, P, DOUT).transpose(1, 0, 2)
        .reshape(P, 2 * DOUT).astype(fp8))

    cv = np.full(2 * P, -1.0, dtype=np.float32)
    cv[:NJ] = np.arange(NJ, dtype=np.float32) // RANK
    cvals = np.ascontiguousarray(cv.reshape(2, P).T)            # [128, 2]

    in_maps = []
    for c in range(N_CORES):
        sl = slice(c * BPC, (c + 1) * BPC)
        xs = np.zeros((TOKP, DIN), dtype=np.float32)
        xs[:TOK] = x[sl].reshape(TOK, DIN)
        eds = np.zeros((TOKP, DIN), dtype=np.float32)
        eds[:TOK] = ed[sl].reshape(TOK, DIN)
        idxs = np.full(TOKP, -1.0, dtype=np.float32)
        idxs[:TOK] = idx[sl].reshape(TOK).astype(np.float32)
        xsT = np.ascontiguousarray(xs.T)                        # [768, 640]
        edT = np.ascontiguousarray(eds.T)
        # edH[p, i*1280 + h*640 + t] = ed.T[(2i+h)*128+p, t]
        edH = np.ascontiguousarray(
            edT.reshape(KP, 2, P, TOKP).transpose(2, 0, 1, 3)
            .reshape(P, KP * 2 * TOKP).astype(fp8))
        # x8H[p, ti*256 + h*128 + u] = x.T[h*128+p, ti*128+u]
        x8H = np.ascontiguousarray(
            xsT[:KF * P].reshape(KF, P, NT, P).transpose(1, 2, 0, 3)
            .reshape(P, NT * KF * P).astype(fp8))
        in_maps.append({
            "xT": np.ascontiguousarray(xsT.astype(bf16)),
            "x8H": x8H,
            "edH": edH,
            "idxf": np.ascontiguousarray(idxs.reshape(1, TOKP)),
            "cvals": cvals,
            "WT": WT,
            "W8H": W8H,
            "ldH": ldH,
            "luH": luH,
        })
    return in_maps


def kernel(x, edit_direction, concept_idx, lora_down, lora_up, W, b_lin,
           _trace=False, **_ignored):
    nc = get_bass()
    in_maps = make_in_maps(x, edit_direction, concept_idx, lora_down, lora_up,
                           W, b_lin)
    res = run_bass_kernel_spmd(nc, in_maps, core_ids=list(range(N_CORES)),
                               trace=_trace)
    out = np.concatenate([np.asarray(r["out"][:TOK], dtype=np.float32)
                          for r in res.results], axis=0)
    out = out.reshape(B, T, DOUT)
    if _trace:
        kernel.last_results = res
    return out
